# revision 14
# baseline (speedup 1.0000x reference)
"""GroupedQueryAttention kernel for 8 Trainium2 NeuronCores.

Sharding: core c = (batch b = c//2, seq-half sh = c%2). Each core computes the
full attention output for 1024 query rows of one batch: all 8 q heads
(2 kv heads), plus the q/k/v projections and the o-projection for those rows.

All matmul operands are bf16 (fp32 matmuls run as two PE passes on TRN2 —
bf16 halves tensor-engine time); accumulation stays fp32 in PSUM and the
softmax denominator/reciprocal stays fp32.

The attention inner loop is paced by the scalar engine's exp throughput
(1 elem/cycle/lane, ~1.06us per [128,1024] tile); the tensor engine runs
just under that pace. Inputs stream in chunks ordered by first use; V is
projected in transposed layout (512-wide matmuls) and moved to natural
layout by the otherwise-idle DMA xbar transpose engine; qc0's o-projection
rides in qc1's tensor-engine slack; scores triple-buffer in PSUM so the exp
stream tolerates short tensor-engine excursions.

On-device layout: scoresT [keys, queries] so softmax-exp'd probabilities feed
attn@v matmuls directly as the moving operand (no transposes anywhere).
Denominators are produced by ones-weight matmuls replicated across all 64
output partitions, so normalization is a plain elementwise multiply.
"""

import numpy as np

B, S, D = 4, 2048, 512
H, KV, DH = 8, 2, 64
SQ = S // 2  # queries per core
NCORES = 8
PAIRS = 4  # head pairs (p, p+4); p -> kv0 rows 0:64, p+4 -> kv1 rows 64:128
SCALE = 1.0 / 8.0  # 1/sqrt(DH)
PERM = [0, 4, 1, 5, 2, 6, 3, 7]  # head order: pair-major
NKB = S // 128  # 16 key blocks

_built = {}


def _build_nc():
    import concourse.mybir as mybir
    import concourse.tile as tile
    from concourse import bacc

    fp32 = mybir.dt.float32
    bf16 = mybir.dt.bfloat16
    Exp = mybir.ActivationFunctionType.Exp

    nc = bacc.Bacc("TRN2", target_bir_lowering=False, debug=False,
                   num_devices=NCORES)

    xT = nc.dram_tensor("xT", [D, S], bf16, kind="ExternalInput").ap()
    wq = nc.dram_tensor("wq", [D, D], bf16, kind="ExternalInput").ap()
    wk = nc.dram_tensor("wk", [D, KV * DH], bf16, kind="ExternalInput").ap()
    wv = nc.dram_tensor("wv", [D, KV * DH], bf16, kind="ExternalInput").ap()
    wo = nc.dram_tensor("wo", [D, D], bf16, kind="ExternalInput").ap()
    bqp = nc.dram_tensor("bqp", [128, PAIRS], fp32, kind="ExternalInput").ap()
    bkvp = nc.dram_tensor("bkvp", [128, 1], fp32, kind="ExternalInput").ap()
    bvp = nc.dram_tensor("bvp", [128, 1], fp32, kind="ExternalInput").ap()
    bobc = nc.dram_tensor("bobc", [128, D], fp32, kind="ExternalInput").ap()
    y = nc.dram_tensor("y", [SQ, D], bf16, kind="ExternalOutput").ap()

    with tile.TileContext(nc) as tc:
        with (
            tc.tile_pool(name="consts", bufs=1) as consts,
            tc.tile_pool(name="epool", bufs=3) as epool,
            tc.tile_pool(name="opool", bufs=8) as opool,
            tc.tile_pool(name="rpool", bufs=2) as rpool,
            tc.tile_pool(name="ypool", bufs=3) as ypool,
            tc.tile_pool(name="pssc", bufs=3, space="PSUM") as pssc,
            tc.tile_pool(name="ps512", bufs=2, space="PSUM") as ps512,
        ):
            # ---- input DMAs, ordered by first use ----
            xT_r = xT.rearrange("(c p) s -> p c s", p=128)
            xt_c = [consts.tile([128, 4, 512], bf16, name=f"xt{i}",
                                tag=f"xt{i}") for i in range(4)]

            wk_sb = consts.tile([128, 4, 128], bf16, tag="wk")
            nc.sync.dma_start(wk_sb[:], wk.rearrange("(c p) j -> p c j", p=128))
            bkv_sb = consts.tile([128, 1], fp32, tag="bkv")
            nc.sync.dma_start(bkv_sb[:], bkvp)
            nc.sync.dma_start(xt_c[0][:], xT_r[:, :, 0:512])
            wq_sb = consts.tile([128, 4, D], bf16, tag="wq")
            nc.sync.dma_start(wq_sb[:], wq.rearrange("(c p) j -> p c j", p=128))
            bq_sb = consts.tile([128, PAIRS], fp32, tag="bq")
            nc.sync.dma_start(bq_sb[:], bqp)
            wv_sb = consts.tile([128, 4, 128], bf16, tag="wv")
            nc.sync.dma_start(wv_sb[:], wv.rearrange("(c p) j -> p c j", p=128))
            bv_sb = consts.tile([128, 1], fp32, tag="bv")
            nc.sync.dma_start(bv_sb[:], bvp)
            nc.sync.dma_start(xt_c[1][:], xT_r[:, :, 512:1024])
            nc.sync.dma_start(xt_c[2][:], xT_r[:, :, 1024:1536])
            nc.sync.dma_start(xt_c[3][:], xT_r[:, :, 1536:2048])
            wo_sb = consts.tile([128, 4, D], bf16, tag="wo")
            nc.sync.dma_start(wo_sb[:], wo.rearrange("(c p) j -> p c j", p=128))
            bo_sb = consts.tile([128, D], fp32, tag="bo")
            nc.sync.dma_start(bo_sb[:], bobc)
            ones_sb = consts.tile([128, DH], bf16, tag="ones")
            nc.vector.memset(ones_sb[:], 1.0)

            # ---- projection emitters (invoked interleaved, see below) ----
            kt_c = [consts.tile([128, 512], bf16, name=f"kt{i}", tag=f"kt{i}")
                    for i in range(4)]
            vt_c = [consts.tile([128, 512], bf16, name=f"vt{i}", tag=f"vt{i}")
                    for i in range(4)]
            v_c = [consts.tile([128, 4, 128], bf16, name=f"v{i}", tag=f"v{i}")
                   for i in range(4)]
            qt_c = {}

            def kt_chunk(sc):
                # kT [128 (kv0|kv1 head-dim), 512 keys]
                ps = ps512.tile([128, 512], fp32, tag="ps512")
                for c in range(4):
                    nc.tensor.matmul(ps[:], wk_sb[:, c, :], xt_c[sc][:, c, :],
                                     start=(c == 0), stop=(c == 3))
                nc.vector.tensor_scalar_add(kt_c[sc][:], ps[:], bkv_sb[:, 0:1])

            def v_chunk(sc):
                # vT like kT (512-wide matmuls), then DMA-xbar transpose into
                # natural [s-row, head-dim] blocks for the attn@v stationary
                ps = ps512.tile([128, 512], fp32, tag="ps512")
                for c in range(4):
                    nc.tensor.matmul(ps[:], wv_sb[:, c, :], xt_c[sc][:, c, :],
                                     start=(c == 0), stop=(c == 3))
                nc.vector.tensor_scalar_add(vt_c[sc][:], ps[:], bv_sb[:, 0:1])
                for sb in range(4):
                    nc.sync.dma_start_transpose(
                        v_c[sc][:, sb, :],
                        vt_c[sc][:, sb * 128:(sb + 1) * 128])

            def qt_chunk(pr, sc):
                # qT [128 (head p | head p+4), 512 queries]
                t = qt_c[(pr, sc)] = consts.tile([128, 512], bf16,
                                                 name=f"qt{pr}_{sc}",
                                                 tag=f"qt{pr}_{sc}")
                ps = ps512.tile([128, 512], fp32, tag="ps512")
                for c in range(4):
                    nc.tensor.matmul(ps[:],
                                     wq_sb[:, c, pr * 128:(pr + 1) * 128],
                                     xt_c[sc][:, c, :],
                                     start=(c == 0), stop=(c == 3))
                nc.vector.tensor_scalar_add(t[:], ps[:], bq_sb[:, pr:pr + 1])

            # ---- attention (qc = query 512-chunk, pr = head pair) ----
            def attention_pair(qc, pr):
                acc = ps512.tile([128, 512], fp32, tag="ps512")
                den = ps512.tile([128, 512], fp32, tag="ps512")
                e_tiles = [None] * NKB
                qt0 = qt_c[(pr, qc)]

                def attnv(kb):
                    e = e_tiles[kb]
                    vt = v_c[kb // 4]
                    sb = kb % 4
                    nc.tensor.matmul(acc[0:64, :], vt[:, sb, 0:64],
                                     e[:, 0:512],
                                     start=(kb == 0), stop=(kb == NKB - 1),
                                     tile_position=(0, 0))
                    nc.tensor.matmul(acc[64:128, :], vt[:, sb, 64:128],
                                     e[:, 512:1024],
                                     start=(kb == 0), stop=(kb == NKB - 1),
                                     tile_position=(0, 64))
                    nc.tensor.matmul(den[0:64, :], ones_sb[:],
                                     e[:, 0:512],
                                     start=(kb == 0), stop=(kb == NKB - 1),
                                     tile_position=(0, 0))
                    nc.tensor.matmul(den[64:128, :], ones_sb[:],
                                     e[:, 512:1024],
                                     start=(kb == 0), stop=(kb == NKB - 1),
                                     tile_position=(0, 64))

                for kb in range(NKB):
                    kt = kt_c[kb // 4]
                    kcol = (kb % 4) * 128
                    sc_ps = pssc.tile([128, 1024], fp32, tag="scores")
                    nc.tensor.matmul(sc_ps[:, 0:512],
                                     kt[0:64, kcol:kcol + 128], qt0[0:64, :])
                    nc.tensor.matmul(sc_ps[:, 512:1024],
                                     kt[64:128, kcol:kcol + 128],
                                     qt0[64:128, :])
                    e = epool.tile([128, 1024], bf16, tag="E")
                    e_tiles[kb] = e
                    nc.scalar.activation(e[:], sc_ps[:], Exp, scale=SCALE)
                    # software pipeline: consume previous block's probs so
                    # PE never waits on the exp of the current block
                    if kb >= 1:
                        attnv(kb - 1)
                attnv(NKB - 1)

                rb = rpool.tile([128, 512], fp32, tag="recip")
                scr = rpool.tile([128, 512], fp32, tag="rscr")
                nc.vector.reciprocal_approx_accurate(rb[:], den[:], scr[:])
                ot = opool.tile([128, 512], bf16, tag="outT")
                nc.vector.tensor_mul(ot[:], acc[:], rb[:])
                return ot

            def o_proj_group(ots, qc, m):
                yp = ps512.tile([128, 512], fp32, tag="ps512")
                for pr2 in range(PAIRS):
                    nc.tensor.matmul(yp[:], ots[pr2][:, m * 128:(m + 1) * 128],
                                     wo_sb[:, pr2, :],
                                     start=(pr2 == 0), stop=(pr2 == 3))
                yt = ypool.tile([128, 512], bf16, tag="y")
                nc.vector.tensor_add(yt[:], yp[:], bo_sb[:])
                blk = qc * 4 + m
                nc.sync.dma_start(y[blk * 128:(blk + 1) * 128, :], yt[:])

            kt_chunk(0)
            for pr in range(PAIRS):
                qt_chunk(pr, 0)
            v_chunk(0)
            kt_chunk(1)
            for pr in range(PAIRS):
                qt_chunk(pr, 1)
            v_chunk(1)
            kt_chunk(2)
            v_chunk(2)
            kt_chunk(3)
            v_chunk(3)

            ots0 = [attention_pair(0, pr) for pr in range(PAIRS)]
            ots1 = []
            for pr in range(PAIRS):
                ots1.append(attention_pair(1, pr))
                # qc0's o-projection rides in qc1's tensor-engine slack
                o_proj_group(ots0, 0, pr)
            for m in range(4):
                o_proj_group(ots1, 1, m)

    nc.finalize()
    return nc


def _get_nc():
    if "nc" not in _built:
        _built["nc"] = _build_nc()
    return _built["nc"]


def _prep_in_maps(x, Wq, bq, Wk, bk, Wv, bv, Wo, bo):
    import ml_dtypes

    bf16 = ml_dtypes.bfloat16

    x = np.ascontiguousarray(np.asarray(x, np.float32))
    Wq = np.asarray(Wq, np.float32)
    bq = np.asarray(bq, np.float32)
    Wk = np.asarray(Wk, np.float32)
    bk = np.asarray(bk, np.float32)
    Wv = np.asarray(Wv, np.float32)
    bv = np.asarray(bv, np.float32)
    Wo = np.asarray(Wo, np.float32)
    bo = np.asarray(bo, np.float32)

    wq_p = np.ascontiguousarray(
        Wq.reshape(D, H, DH)[:, PERM, :].reshape(D, D).astype(bf16))
    wo_p = np.ascontiguousarray(
        Wo.reshape(H, DH, D)[PERM].reshape(D, D).astype(bf16))
    wk_b = np.ascontiguousarray(Wk.astype(bf16))
    wv_b = np.ascontiguousarray(Wv.astype(bf16))
    bq_p = np.ascontiguousarray(
        bq.reshape(H, DH)[PERM].reshape(PAIRS, 128).T)
    bkv_p = np.ascontiguousarray(bk.reshape(128, 1))
    bv_p = np.ascontiguousarray(bv.reshape(128, 1))
    bo_bc = np.ascontiguousarray(np.tile(bo[None, :], (128, 1)))

    in_maps = []
    for c in range(NCORES):
        b, sh = divmod(c, 2)
        xroll = np.roll(x[b], -sh * SQ, axis=0)
        in_maps.append({
            "xT": np.ascontiguousarray(xroll.T.astype(bf16)),
            "wq": wq_p, "wk": wk_b, "wv": wv_b, "wo": wo_p,
            "bqp": bq_p, "bkvp": bkv_p, "bvp": bv_p, "bobc": bo_bc,
        })
    return in_maps


def kernel(x, Wq, bq, Wk, bk, Wv, bv, Wo, bo):
    from concourse.bass_utils import run_bass_kernel_spmd

    in_maps = _prep_in_maps(x, Wq, bq, Wk, bk, Wv, bv, Wo, bo)
    nc = _get_nc()
    res = run_bass_kernel_spmd(nc, in_maps, list(range(NCORES)))
    out = np.empty((B, S, D), np.float32)
    for c in range(NCORES):
        b, sh = divmod(c, 2)
        out[b, sh * SQ:(sh + 1) * SQ, :] = \
            np.asarray(res.results[c]["y"]).astype(np.float32)
    return out


# revision 16
# speedup vs baseline: 1.1879x; 1.1879x over previous
"""GroupedQueryAttention kernel for 8 Trainium2 NeuronCores.

Sharding: core c = (batch b = c//2, seq-half sh = c%2). Each core computes the
full attention output for 1024 query rows of one batch: all 8 q heads
(2 kv heads), plus the q/k/v projections and the o-projection for those rows.

All matmul operands are bf16 (fp32 matmuls run as two PE passes on TRN2 —
bf16 halves tensor-engine time); accumulation stays fp32 in PSUM and the
softmax denominator/reciprocal stays fp32.

The attention inner loop is paced by the scalar engine's exp throughput
(1 elem/cycle/lane, ~1.06us per [128,1024] tile); the tensor engine runs
just under that pace. Inputs stream in chunks ordered by first use; V is
projected in transposed layout (512-wide matmuls) and moved to natural
layout by the otherwise-idle DMA xbar transpose engine; qc0's o-projection
rides in qc1's tensor-engine slack; scores triple-buffer in PSUM so the exp
stream tolerates short tensor-engine excursions.

On-device layout: scoresT [keys, queries] so softmax-exp'd probabilities feed
attn@v matmuls directly as the moving operand (no transposes anywhere).
Denominators are produced by ones-weight matmuls replicated across all 64
output partitions, so normalization is a plain elementwise multiply.
"""

import numpy as np

B, S, D = 4, 2048, 512
H, KV, DH = 8, 2, 64
SQ = S // 2  # queries per core
NCORES = 8
PAIRS = 4  # head pairs (p, p+4); p -> kv0 rows 0:64, p+4 -> kv1 rows 64:128
SCALE = 1.0 / 8.0  # 1/sqrt(DH)
PERM = [0, 4, 1, 5, 2, 6, 3, 7]  # head order: pair-major
NKB = S // 128  # 16 key blocks

_built = {}


def _build_nc():
    import concourse.mybir as mybir
    import concourse.tile as tile
    from concourse import bacc

    fp32 = mybir.dt.float32
    bf16 = mybir.dt.bfloat16
    Exp = mybir.ActivationFunctionType.Exp

    nc = bacc.Bacc("TRN2", target_bir_lowering=False, debug=False,
                   num_devices=NCORES)

    xT = nc.dram_tensor("xT", [D, S], bf16, kind="ExternalInput").ap()
    wq = nc.dram_tensor("wq", [D, D], bf16, kind="ExternalInput").ap()
    wk = nc.dram_tensor("wk", [D, KV * DH], bf16, kind="ExternalInput").ap()
    wv = nc.dram_tensor("wv", [D, KV * DH], bf16, kind="ExternalInput").ap()
    wo = nc.dram_tensor("wo", [D, D], bf16, kind="ExternalInput").ap()
    bqp = nc.dram_tensor("bqp", [128, PAIRS], fp32, kind="ExternalInput").ap()
    bkvp = nc.dram_tensor("bkvp", [128, 1], fp32, kind="ExternalInput").ap()
    bvp = nc.dram_tensor("bvp", [128, 1], fp32, kind="ExternalInput").ap()
    bobc = nc.dram_tensor("bobc", [128, D], fp32, kind="ExternalInput").ap()
    y = nc.dram_tensor("y", [SQ, D], bf16, kind="ExternalOutput").ap()

    with tile.TileContext(nc) as tc:
        with (
            tc.tile_pool(name="consts", bufs=1) as consts,
            tc.tile_pool(name="epool", bufs=3) as epool,
            tc.tile_pool(name="opool", bufs=8) as opool,
            tc.tile_pool(name="rpool", bufs=2) as rpool,
            tc.tile_pool(name="ypool", bufs=3) as ypool,
            tc.tile_pool(name="pssc", bufs=3, space="PSUM") as pssc,
            tc.tile_pool(name="ps512", bufs=2, space="PSUM") as ps512,
        ):
            # ---- input DMAs, ordered by first use ----
            xT_r = xT.rearrange("(c p) s -> p c s", p=128)
            xt_c = [consts.tile([128, 4, 512], bf16, name=f"xt{i}",
                                tag=f"xt{i}") for i in range(4)]

            wk_sb = consts.tile([128, 4, 128], bf16, tag="wk")
            nc.sync.dma_start(wk_sb[:], wk.rearrange("(c p) j -> p c j", p=128))
            bkv_sb = consts.tile([128, 1], fp32, tag="bkv")
            nc.sync.dma_start(bkv_sb[:], bkvp)
            nc.sync.dma_start(xt_c[0][:], xT_r[:, :, 0:512])
            wq_sb = consts.tile([128, 4, D], bf16, tag="wq")
            nc.sync.dma_start(wq_sb[:], wq.rearrange("(c p) j -> p c j", p=128))
            bq_sb = consts.tile([128, PAIRS], fp32, tag="bq")
            nc.sync.dma_start(bq_sb[:], bqp)
            wv_sb = consts.tile([128, 4, 128], bf16, tag="wv")
            nc.sync.dma_start(wv_sb[:], wv.rearrange("(c p) j -> p c j", p=128))
            bv_sb = consts.tile([128, 1], fp32, tag="bv")
            nc.sync.dma_start(bv_sb[:], bvp)
            nc.sync.dma_start(xt_c[1][:], xT_r[:, :, 512:1024])
            nc.sync.dma_start(xt_c[2][:], xT_r[:, :, 1024:1536])
            nc.sync.dma_start(xt_c[3][:], xT_r[:, :, 1536:2048])
            wo_sb = consts.tile([128, 4, D], bf16, tag="wo")
            nc.sync.dma_start(wo_sb[:], wo.rearrange("(c p) j -> p c j", p=128))
            bo_sb = consts.tile([128, D], fp32, tag="bo")
            nc.sync.dma_start(bo_sb[:], bobc)
            ones_sb = consts.tile([128, DH], bf16, tag="ones")
            nc.vector.memset(ones_sb[:], 1.0)

            # ---- projection emitters (invoked interleaved, see below) ----
            kt_c = [consts.tile([128, 512], bf16, name=f"kt{i}", tag=f"kt{i}")
                    for i in range(4)]
            vt_c = [consts.tile([128, 512], bf16, name=f"vt{i}", tag=f"vt{i}")
                    for i in range(4)]
            v_c = [consts.tile([128, 4, 128], bf16, name=f"v{i}", tag=f"v{i}")
                   for i in range(4)]
            qt_c = {}

            def kt_chunk(sc):
                # kT [128 (kv0|kv1 head-dim), 512 keys]
                ps = ps512.tile([128, 512], fp32, tag="ps512")
                for c in range(4):
                    nc.tensor.matmul(ps[:], wk_sb[:, c, :], xt_c[sc][:, c, :],
                                     start=(c == 0), stop=(c == 3))
                nc.vector.tensor_scalar_add(kt_c[sc][:], ps[:], bkv_sb[:, 0:1])

            def v_chunk(sc):
                # vT like kT (512-wide matmuls), then DMA-xbar transpose into
                # natural [s-row, head-dim] blocks for the attn@v stationary
                ps = ps512.tile([128, 512], fp32, tag="ps512")
                for c in range(4):
                    nc.tensor.matmul(ps[:], wv_sb[:, c, :], xt_c[sc][:, c, :],
                                     start=(c == 0), stop=(c == 3))
                nc.vector.tensor_scalar_add(vt_c[sc][:], ps[:], bv_sb[:, 0:1])
                for sb in range(4):
                    nc.sync.dma_start_transpose(
                        v_c[sc][:, sb, :],
                        vt_c[sc][:, sb * 128:(sb + 1) * 128])

            def qt_chunk(pr, sc):
                # qT [128 (head p | head p+4), 512 queries]
                t = qt_c[(pr, sc)] = consts.tile([128, 512], bf16,
                                                 name=f"qt{pr}_{sc}",
                                                 tag=f"qt{pr}_{sc}")
                ps = ps512.tile([128, 512], fp32, tag="ps512")
                for c in range(4):
                    nc.tensor.matmul(ps[:],
                                     wq_sb[:, c, pr * 128:(pr + 1) * 128],
                                     xt_c[sc][:, c, :],
                                     start=(c == 0), stop=(c == 3))
                nc.vector.tensor_scalar_add(t[:], ps[:], bq_sb[:, pr:pr + 1])

            # ---- attention (qc = query 512-chunk, pr = head pair) ----
            def attention_pair(qc, pr):
                acc = ps512.tile([128, 512], fp32, tag="ps512")
                den = ps512.tile([128, 512], fp32, tag="ps512")
                e_tiles = [None] * NKB
                qt0 = qt_c[(pr, qc)]

                def attnv(kb):
                    e = e_tiles[kb]
                    vt = v_c[kb // 4]
                    sb = kb % 4
                    nc.tensor.matmul(acc[0:64, :], vt[:, sb, 0:64],
                                     e[:, 0:512],
                                     start=(kb == 0), stop=(kb == NKB - 1),
                                     tile_position=(0, 0))
                    nc.tensor.matmul(acc[64:128, :], vt[:, sb, 64:128],
                                     e[:, 512:1024],
                                     start=(kb == 0), stop=(kb == NKB - 1),
                                     tile_position=(0, 64))
                    nc.tensor.matmul(den[0:64, :], ones_sb[:],
                                     e[:, 0:512],
                                     start=(kb == 0), stop=(kb == NKB - 1),
                                     tile_position=(0, 0))
                    nc.tensor.matmul(den[64:128, :], ones_sb[:],
                                     e[:, 512:1024],
                                     start=(kb == 0), stop=(kb == NKB - 1),
                                     tile_position=(0, 64))

                for kb in range(NKB):
                    kt = kt_c[kb // 4]
                    kcol = (kb % 4) * 128
                    sc_ps = pssc.tile([128, 1024], fp32, tag="scores")
                    nc.tensor.matmul(sc_ps[:, 0:512],
                                     kt[0:64, kcol:kcol + 128], qt0[0:64, :])
                    nc.tensor.matmul(sc_ps[:, 512:1024],
                                     kt[64:128, kcol:kcol + 128],
                                     qt0[64:128, :])
                    e = epool.tile([128, 1024], bf16, tag="E")
                    e_tiles[kb] = e
                    nc.scalar.activation(e[:], sc_ps[:], Exp, scale=SCALE)
                    # software pipeline: consume previous block's probs so
                    # PE never waits on the exp of the current block
                    if kb >= 1:
                        attnv(kb - 1)
                attnv(NKB - 1)

                rb = rpool.tile([128, 512], fp32, tag="recip")
                scr = rpool.tile([128, 512], fp32, tag="rscr")
                nc.vector.reciprocal_approx_accurate(rb[:], den[:], scr[:])
                ot = opool.tile([128, 512], bf16, tag="outT")
                nc.vector.tensor_mul(ot[:], acc[:], rb[:])
                return ot

            def o_proj_group(ots, qc, m):
                yp = ps512.tile([128, 512], fp32, tag="ps512")
                for pr2 in range(PAIRS):
                    nc.tensor.matmul(yp[:], ots[pr2][:, m * 128:(m + 1) * 128],
                                     wo_sb[:, pr2, :],
                                     start=(pr2 == 0), stop=(pr2 == 3))
                yt = ypool.tile([128, 512], bf16, tag="y")
                nc.vector.tensor_add(yt[:], yp[:], bo_sb[:])
                blk = qc * 4 + m
                nc.sync.dma_start(y[blk * 128:(blk + 1) * 128, :], yt[:])

            kt_chunk(0)
            for pr in range(PAIRS):
                qt_chunk(pr, 0)
            v_chunk(0)
            kt_chunk(1)
            for pr in range(PAIRS):
                qt_chunk(pr, 1)
            v_chunk(1)
            kt_chunk(2)
            v_chunk(2)
            kt_chunk(3)
            v_chunk(3)

            ots0 = [attention_pair(0, pr) for pr in range(PAIRS)]
            ots1 = []
            for pr in range(PAIRS):
                ots1.append(attention_pair(1, pr))
                # qc0's o-projection rides in qc1's tensor-engine slack
                o_proj_group(ots0, 0, pr)
            for m in range(4):
                o_proj_group(ots1, 1, m)

    nc.finalize()
    return nc


def _get_nc():
    if "nc" not in _built:
        _built["nc"] = _build_nc()
    return _built["nc"]


def _prep_in_maps(x, Wq, bq, Wk, bk, Wv, bv, Wo, bo):
    import ml_dtypes

    bf16 = ml_dtypes.bfloat16

    x = np.ascontiguousarray(np.asarray(x, np.float32))
    Wq = np.asarray(Wq, np.float32)
    bq = np.asarray(bq, np.float32)
    Wk = np.asarray(Wk, np.float32)
    bk = np.asarray(bk, np.float32)
    Wv = np.asarray(Wv, np.float32)
    bv = np.asarray(bv, np.float32)
    Wo = np.asarray(Wo, np.float32)
    bo = np.asarray(bo, np.float32)

    wq_p = np.ascontiguousarray(
        Wq.reshape(D, H, DH)[:, PERM, :].reshape(D, D).astype(bf16))
    wo_p = np.ascontiguousarray(
        Wo.reshape(H, DH, D)[PERM].reshape(D, D).astype(bf16))
    wk_b = np.ascontiguousarray(Wk.astype(bf16))
    wv_b = np.ascontiguousarray(Wv.astype(bf16))
    bq_p = np.ascontiguousarray(
        bq.reshape(H, DH)[PERM].reshape(PAIRS, 128).T)
    bkv_p = np.ascontiguousarray(bk.reshape(128, 1))
    bv_p = np.ascontiguousarray(bv.reshape(128, 1))
    bo_bc = np.ascontiguousarray(np.tile(bo[None, :], (128, 1)))

    in_maps = []
    for c in range(NCORES):
        b, sh = divmod(c, 2)
        xroll = np.roll(x[b], -sh * SQ, axis=0)
        in_maps.append({
            "xT": np.ascontiguousarray(xroll.T.astype(bf16)),
            "wq": wq_p, "wk": wk_b, "wv": wv_b, "wo": wo_p,
            "bqp": bq_p, "bkvp": bkv_p, "bvp": bv_p, "bobc": bo_bc,
        })
    return in_maps


def kernel(x, Wq, bq, Wk, bk, Wv, bv, Wo, bo):
    from concourse.bass_utils import run_bass_kernel_spmd

    in_maps = _prep_in_maps(x, Wq, bq, Wk, bk, Wv, bv, Wo, bo)
    nc = _get_nc()
    res = run_bass_kernel_spmd(nc, in_maps, list(range(NCORES)))
    out = np.empty((B, S, D), np.float32)
    for c in range(NCORES):
        b, sh = divmod(c, 2)
        out[b, sh * SQ:(sh + 1) * SQ, :] = \
            np.asarray(res.results[c]["y"]).astype(np.float32)
    return out
